# revision 1
# baseline (speedup 1.0000x reference)
"""DMPNN layer kernel for 8 Trainium2 NeuronCores.

Sharding: data-parallel over destination nodes j (dim 2 of edge_attr/adj,
dim 1 of the output). Each core gets a 64-column j-slice of edge_attr/adj,
the full h (needed because messages sum over all source nodes i), and the
small weights replicated. The batch-global mask (adj.sum(0) > 0) only needs
the core's own j-slice of adj over the full batch, so no collective at all.

Math per core (j in its 64-column slice, source nodes i = 4p + q):
  mask[i,j]   = max_b adj[b,i,j]                    (adj is 0/1)
  deg[j]      = sum_i mask[i,j]
  mh[b,j,f]   = sum_i mask[i,j] h[b,i,f]
  me[b,j,e]   = sum_i mask[i,j] edge[b,i,j,e]
  msg[b,j,o]  = sum_f Wh[o,f] mh[b,j,f] + deg[j] wb[o] + sum_e We[o,e] me[b,j,e]
  out[b,j,o]  = sum_f U[o,f] (h[b,j,f] + msg[b,j,f]) + ub[o]

The two bias terms ride as rank-1 outer-product matmuls accumulated into the
same PSUM tiles, so no ones-columns/memsets are needed in the hot loop.
"""

import numpy as np


def _ensure_path():
    try:
        import concourse.bass  # noqa: F401
    except ImportError:
        import sys

        for p in ("/opt/trn_rl_repo", "/root/.axon_site/_ro/trn_rl_repo"):
            if p not in sys.path:
                sys.path.insert(0, p)


B, N, H, E = 8, 512, 64, 8
NCORES = 8
JB = N // NCORES  # 64 destination columns per core
CH = N // 128  # 4 source-node sub-chunks (i = 4p + q)


_CACHE = {}


def _build_program():
    _ensure_path()
    import concourse.bacc as bacc
    import concourse.mybir as mybir
    import concourse.tile as tile

    dt = mybir.dt
    f32 = dt.float32
    i32 = dt.int32
    Alu = mybir.AluOpType

    nc = bacc.Bacc("TRN2", debug=False, num_devices=NCORES)

    edge = nc.dram_tensor("edge", [B, N, JB, E], f32, kind="ExternalInput").ap()
    adjs = nc.dram_tensor("adjs", [B, N, JB], i32, kind="ExternalInput").ap()
    h = nc.dram_tensor("h", [B, N, H], f32, kind="ExternalInput").ap()
    hs = nc.dram_tensor("hs", [B, JB, H], f32, kind="ExternalInput").ap()
    Ww = nc.dram_tensor("Ww", [H, H + E], f32, kind="ExternalInput").ap()
    Wb = nc.dram_tensor("Wb", [1, H], f32, kind="ExternalInput").ap()
    Uw = nc.dram_tensor("Uw", [H, H], f32, kind="ExternalInput").ap()
    Ub = nc.dram_tensor("Ub", [1, H], f32, kind="ExternalInput").ap()
    out = nc.dram_tensor("out", [B, H, JB], f32, kind="ExternalOutput").ap()

    ident_d = nc.inline_tensor(np.eye(128, dtype=np.float32), "ident")
    # DRAM bounce for the (j,e) -> [e, j] partition remap of the reduced
    # edge messages (PSUM cannot be DMA'd and engines cannot cross partitions).
    me_bounce = nc.dram_tensor("me_bounce", [B, JB * E], f32).ap()

    with tile.TileContext(nc) as tc:
        with (
            tc.tile_pool(name="const", bufs=1) as cpool,
            tc.tile_pool(name="edge", bufs=4) as epool,
            tc.tile_pool(name="masked", bufs=3) as mpool,
            tc.tile_pool(name="small", bufs=3) as spool,
            tc.tile_pool(name="pe", bufs=2, space="PSUM") as ppool_e,
            tc.tile_pool(name="pmh", bufs=2, space="PSUM") as ppool_mh,
            tc.tile_pool(name="pmsg", bufs=2, space="PSUM") as ppool_msg,
            tc.tile_pool(name="pout", bufs=2, space="PSUM") as ppool_out,
        ):
            # ---------------- mask first: it gates the whole pipeline -----
            adj_sb = cpool.tile([128, B * CH * JB], i32)
            adj_v = adj_sb.rearrange("p (b qj) -> p b qj", b=B)
            for b in range(B):
                nc.sync.dma_start(
                    out=adj_v[:, b],
                    in_=adjs[b].rearrange("(p q) j -> p (q j)", q=CH),
                )
            # pairwise max tree over the batch axis (adj is 0/1 so
            # max == (sum > 0)); overlaps with per-batch adj DMA arrival
            mt0 = cpool.tile([128, CH * JB], i32, name="mt0")
            mt1 = cpool.tile([128, CH * JB], i32, name="mt1")
            mt2 = cpool.tile([128, CH * JB], i32, name="mt2")
            mt3 = cpool.tile([128, CH * JB], i32, name="mt3")
            nc.vector.tensor_tensor(mt0[:, :], adj_v[:, 0], adj_v[:, 1], Alu.max)
            nc.vector.tensor_tensor(mt1[:, :], adj_v[:, 2], adj_v[:, 3], Alu.max)
            nc.vector.tensor_tensor(mt2[:, :], adj_v[:, 4], adj_v[:, 5], Alu.max)
            nc.vector.tensor_tensor(mt3[:, :], adj_v[:, 6], adj_v[:, 7], Alu.max)
            nc.vector.tensor_tensor(mt0[:, :], mt0[:, :], mt1[:, :], Alu.max)
            nc.vector.tensor_tensor(mt2[:, :], mt2[:, :], mt3[:, :], Alu.max)
            mask = cpool.tile([128, CH * JB], f32)
            nc.vector.tensor_tensor(mask[:, :], mt0[:, :], mt2[:, :], Alu.max)

            # ---------------- bulk node features (one DMA each) ----------
            # h_all[p, (b q f)]: h[b, 4p+q, f] — 1 KiB contiguous runs
            h_all = cpool.tile([128, B * CH * H], f32)
            nc.sync.dma_start(
                out=h_all.rearrange("p (b qf) -> p b qf", b=B),
                in_=h.rearrange("b (p q) f -> p b (q f)", q=CH),
            )
            # hs_all[j, (b f)]: h[b, j0+j, f]
            hs_all = cpool.tile([JB, B * H], f32)
            nc.sync.dma_start(
                out=hs_all.rearrange("j (b f) -> j b f", b=B),
                in_=hs.rearrange("b j f -> j b f"),
            )

            # ---------------- constants ----------------
            ident = cpool.tile([128, 128], f32)
            nc.scalar.dma_start(out=ident[:, :], in_=ident_d.ap()[:, :])
            ones_stat = cpool.tile([128, 1], f32)
            nc.vector.memset(ones_stat[:, :], 1.0)
            ones_row = cpool.tile([1, JB], f32)
            nc.vector.memset(ones_row[:, :], 1.0)

            Ww_sb = cpool.tile([H, H + E], f32)
            nc.scalar.dma_start(out=Ww_sb[:, :], in_=Ww[:, :])
            Uw_sb = cpool.tile([H, H], f32)
            nc.scalar.dma_start(out=Uw_sb[:, :], in_=Uw[:, :])
            wb_sb = cpool.tile([1, H], f32)
            nc.scalar.dma_start(out=wb_sb[:, :], in_=Wb[:, :])
            ub_sb = cpool.tile([1, H], f32)
            nc.scalar.dma_start(out=ub_sb[:, :], in_=Ub[:, :])

            # Wh2 = Wh^T (64x64), WeM = We^T (8x64), U2 = U^T (64x64)
            Wh2 = cpool.tile([H, H], f32)
            WeM = cpool.tile([E, H], f32)
            U2 = cpool.tile([H, H], f32)

            pwh = ppool_out.tile([H, H], f32, tag="o", name="pwh")
            nc.tensor.transpose(pwh[:, :], Ww_sb[:, 0:H], ident[0:H, 0:H])
            nc.vector.tensor_copy(Wh2[:, :], pwh[:, :])

            pwe = ppool_out.tile([E, H], f32, tag="o", name="pwe")
            nc.tensor.transpose(pwe[:, :], Ww_sb[:, H : H + E], ident[0:H, 0:H])
            nc.vector.tensor_copy(WeM[:, :], pwe[:, :])

            puw = ppool_out.tile([H, H], f32, tag="o", name="puw")
            nc.tensor.transpose(puw[:, :], Uw_sb[:, :], ident[0:H, 0:H])
            nc.vector.tensor_copy(U2[:, :], puw[:, :])

            # hs transposed: hsT_all[f, (b j)]
            hsT_all = cpool.tile([H, B * JB], f32)
            for b in range(B):
                pht = ppool_msg.tile([H, JB], f32, tag="m", name="pht")
                nc.tensor.transpose(
                    pht[:, :], hs_all[:, b * H : (b + 1) * H], ident[0:JB, 0:JB]
                )
                nc.vector.tensor_copy(hsT_all[:, b * JB : (b + 1) * JB], pht[:, :])

            # deg as a row vector [1, j] = ones^T @ mask
            psum_deg = ppool_out.tile([1, JB], f32, tag="o", name="psum_deg")
            for c in range(CH):
                nc.tensor.matmul(
                    psum_deg[:, :],
                    lhsT=ones_stat[:, :],
                    rhs=mask[:, c * JB : (c + 1) * JB],
                    start=(c == 0),
                    stop=(c == CH - 1),
                )
            deg_sb = cpool.tile([1, JB], f32)
            nc.scalar.copy(deg_sb[:, :], psum_deg[:, :])

            # broadcast view of the mask over the e axis (stride-0)
            mask_bcast = mask.rearrange("p (q j) -> p q j", q=CH).broadcast_to(
                [128, CH, JB, E]
            )

            # ---------------- per-batch software pipeline ----------------
            # Three stages emitted with lags so the per-engine FIFO queues
            # never couple batch b's slow tail (PSUM extraction + the DRAM
            # bounce round-trip) to batch b+1's head.
            st = [dict() for _ in range(B)]

            def s1(b):
                # heavy streaming: edge load, mask multiply, the two
                # i-contraction matmul groups
                d = st[b]
                edge_t = epool.tile([128, CH * JB * E], f32, name="edge_t")
                nc.sync.dma_start(
                    out=edge_t[:, :],
                    in_=edge[b].rearrange("(p q) j e -> p (q j e)", q=CH),
                )
                masked = mpool.tile([128, CH * JB * E], f32, name="masked")
                mk_v = masked.rearrange("p (q j e) -> p q j e", q=CH, j=JB)
                eg_v = edge_t.rearrange("p (q j e) -> p q j e", q=CH, j=JB)
                psum_e = ppool_e.tile([1, JB * E], f32, name="psum_e")
                for c in range(CH):
                    nc.vector.tensor_tensor(
                        out=mk_v[:, c],
                        in0=eg_v[:, c],
                        in1=mask_bcast[:, c],
                        op=Alu.mult,
                    )
                    nc.tensor.matmul(
                        psum_e[:, :],
                        lhsT=ones_stat[:, :],
                        rhs=masked[:, c * JB * E : (c + 1) * JB * E],
                        start=(c == 0),
                        stop=(c == CH - 1),
                    )
                d["psum_e"] = psum_e

                psum_mhT = ppool_mh.tile([H, JB], f32, name="psum_mhT")
                for c in range(CH):
                    nc.tensor.matmul(
                        psum_mhT[:, :],
                        lhsT=h_all[:, (b * CH + c) * H : (b * CH + c + 1) * H],
                        rhs=mask[:, c * JB : (c + 1) * JB],
                        start=(c == 0),
                        stop=(c == CH - 1),
                    )
                d["psum_mhT"] = psum_mhT

            def s2(b):
                # PSUM extraction + the (j,e) -> [e,j] remap round-trip
                d = st[b]
                me_sb = spool.tile([1, JB * E], f32, name="me_sb")
                nc.scalar.copy(me_sb[:, :], d["psum_e"][:, :])
                nc.scalar.dma_start(out=me_bounce[b : b + 1], in_=me_sb[0:1, :])
                me_T = spool.tile([E, JB], f32, name="me_T")
                nc.scalar.dma_start(
                    out=me_T[:, :],
                    in_=me_bounce[b].rearrange("(j e) -> e j", e=E),
                )
                d["me_T"] = me_T

                mhT_s = spool.tile([H, JB], f32, name="mhT_s")
                nc.scalar.copy(mhT_s[:, :], d["psum_mhT"][:, :])
                d["mhT_s"] = mhT_s

            def s3(b):
                # messages + update + output
                d = st[b]
                psum_msgT = ppool_msg.tile([H, JB], f32, tag="m", name="psum_msgT")
                nc.tensor.matmul(
                    psum_msgT[:, :], lhsT=Wh2[:, :], rhs=d["mhT_s"][:, :],
                    start=True, stop=False,
                )
                nc.tensor.matmul(
                    psum_msgT[:, :], lhsT=WeM[:, :], rhs=d["me_T"][:, :],
                    start=False, stop=False,
                )
                nc.tensor.matmul(
                    psum_msgT[:, :], lhsT=wb_sb[:, :], rhs=deg_sb[:, :],
                    start=False, stop=True,
                )
                XT_s = spool.tile([H, JB], f32, name="XT_s")
                nc.vector.tensor_tensor(
                    out=XT_s[:, :],
                    in0=psum_msgT[:, :],
                    in1=hsT_all[:, b * JB : (b + 1) * JB],
                    op=Alu.add,
                )
                psum_outT = ppool_out.tile([H, JB], f32, tag="o", name="psum_outT")
                nc.tensor.matmul(
                    psum_outT[:, :], lhsT=U2[:, :], rhs=XT_s[:, :],
                    start=True, stop=False,
                )
                nc.tensor.matmul(
                    psum_outT[:, :], lhsT=ub_sb[:, :], rhs=ones_row[:, :],
                    start=False, stop=True,
                )
                out_sb = spool.tile([H, JB], f32, name="out_sb")
                nc.scalar.copy(out_sb[:, :], psum_outT[:, :])
                nc.sync.dma_start(out=out[b], in_=out_sb[:, :])

            LAG2, LAG3 = 1, 2
            for b in range(B + LAG3):
                if b < B:
                    s1(b)
                if LAG2 <= b < B + LAG2:
                    s2(b - LAG2)
                if LAG3 <= b:
                    s3(b - LAG3)

    nc.compile()
    return nc


def _get_program():
    if "nc" not in _CACHE:
        _CACHE["nc"] = _build_program()
    return _CACHE["nc"]


def _make_in_maps(h, edge_attr, adj, W_w, W_b, U_w, U_b):
    h = np.ascontiguousarray(np.asarray(h, dtype=np.float32))
    edge_attr = np.asarray(edge_attr, dtype=np.float32)
    adj = np.asarray(adj, dtype=np.int32)
    W_w = np.ascontiguousarray(np.asarray(W_w, dtype=np.float32))
    W_b = np.ascontiguousarray(np.asarray(W_b, dtype=np.float32)).reshape(1, H)
    U_w = np.ascontiguousarray(np.asarray(U_w, dtype=np.float32))
    U_b = np.ascontiguousarray(np.asarray(U_b, dtype=np.float32)).reshape(1, H)

    in_maps = []
    for c in range(NCORES):
        j0 = c * JB
        in_maps.append(
            {
                "edge": np.ascontiguousarray(edge_attr[:, :, j0 : j0 + JB, :]),
                "adjs": np.ascontiguousarray(adj[:, :, j0 : j0 + JB]),
                "h": h,
                "hs": np.ascontiguousarray(h[:, j0 : j0 + JB, :]),
                "Ww": W_w,
                "Wb": W_b,
                "Uw": U_w,
                "Ub": U_b,
            }
        )
    return in_maps


def _install_ntff_hook():
    """The agent image lacks antenv.axon_hooks; synthesize it so trace=True
    can reach the libaxon NTFF profiling entry points."""
    import sys
    import types

    try:
        from antenv.axon_hooks import get_axon_ntff_profile_hook  # noqa: F401

        return
    except ImportError:
        pass
    import antenv

    mod = types.ModuleType("antenv.axon_hooks")
    _h = [None]
    mod.set_axon_ntff_profile_hook = lambda hook: _h.__setitem__(0, hook)
    mod.get_axon_ntff_profile_hook = lambda: _h[0]
    sys.modules["antenv.axon_hooks"] = mod
    antenv.axon_hooks = mod
    try:
        from trn_agent_boot.trn_boot import _ntff_profile_via_ctypes

        mod.set_axon_ntff_profile_hook(
            _ntff_profile_via_ctypes("/opt/axon/libaxon_pjrt.so")
        )
    except Exception:
        pass
    # avoid the bucket upload (no bucket in this container)
    import concourse.bass_utils as bu

    bu.upload_artifacts = lambda tmpdir: str(tmpdir)


def run(h, edge_attr, adj, W_w, W_b, U_w, U_b, trace=False, trace_cores=None):
    """Run the kernel; returns (output, BassKernelResults)."""
    _ensure_path()
    if trace:
        _install_ntff_hook()
    from concourse.bass_utils import run_bass_kernel_spmd

    nc = _get_program()
    in_maps = _make_in_maps(h, edge_attr, adj, W_w, W_b, U_w, U_b)
    kw = {}
    if trace:
        kw = {"trace": True, "trace_cores": trace_cores or [0]}
    res = run_bass_kernel_spmd(nc, in_maps, list(range(NCORES)), **kw)
    outs = [res.results[c]["out"].transpose(0, 2, 1) for c in range(NCORES)]
    full = np.concatenate(outs, axis=1)  # [B, N, H]
    return full, res


def kernel(h, edge_attr, adj, W_w, W_b, U_w, U_b):
    full, _ = run(h, edge_attr, adj, W_w, W_b, U_w, U_b)
    return full



# revision 2
# speedup vs baseline: 1.6186x; 1.6186x over previous
"""DMPNN layer kernel for 8 Trainium2 NeuronCores.

Sharding: data-parallel over destination nodes j (dim 2 of edge_attr/adj,
dim 1 of the output). Each core gets a 64-column j-slice of edge_attr/adj,
the full h (needed because messages sum over all source nodes i), and the
small weights replicated. The batch-global mask (adj.sum(0) > 0) only needs
the core's own j-slice of adj over the full batch, so no collective at all.

Numerics: the correctness gate is rel_err < 2e-2 (norm-relative); staging
edge_attr/h/weights as bf16 keeps the result at ~3e-3 while halving HBM
traffic and doubling PE/DVE throughput. adj is 0/1 so it ships as uint8
(lossless). All matmuls accumulate in fp32 PSUM; the mask multiply is
exact (mask is 0.0/1.0 in bf16).

Math per core (j in its 64-column slice, source nodes i = 4p + q):
  mask[i,j]   = max_b adj[b,i,j]                    (adj is 0/1)
  deg[j]      = sum_i mask[i,j]
  mh[b,j,f]   = sum_i mask[i,j] h[b,i,f]
  me[b,j,e]   = sum_i mask[i,j] edge[b,i,j,e]
  msg[b,j,o]  = sum_f Wh[o,f] mh[b,j,f] + deg[j] wb[o] + sum_e We[o,e] me[b,j,e]
  out[b,j,o]  = sum_f U[o,f] (h[b,j,f] + msg[b,j,f]) + ub[o]

Host pre-arranges all layouts (transposed weights, [p,b,q,f] h, [p,b,q,e,j]
edge) so the device does no transposes; ub rides as a per-partition ACT
bias, wb*deg as a rank-1 matmul. The (j,e)->[e,j] partition remap of the
reduced edge messages goes through a small DRAM bounce (PSUM cannot be
DMA'd and engines cannot cross partitions).
"""

import numpy as np


def _ensure_path():
    try:
        import concourse.bass  # noqa: F401
    except ImportError:
        import sys

        for p in ("/opt/trn_rl_repo", "/root/.axon_site/_ro/trn_rl_repo"):
            if p not in sys.path:
                sys.path.insert(0, p)


B, N, H, E = 8, 512, 64, 8
NCORES = 8
JB = N // NCORES  # 64 destination columns per core
CH = N // 128  # 4 source-node sub-chunks (i = 4p + q)

_CACHE = {}


def _build_program():
    _ensure_path()
    import concourse.bacc as bacc
    import concourse.mybir as mybir
    import concourse.tile as tile

    dt = mybir.dt
    f32 = dt.float32
    bf16 = dt.bfloat16
    u8 = dt.uint8
    Alu = mybir.AluOpType
    ActFn = mybir.ActivationFunctionType

    nc = bacc.Bacc("TRN2", debug=False, num_devices=NCORES)

    # (p, b, q, e, j) — per-partition contiguous 4 KiB per batch
    edge = nc.dram_tensor("edge", [128, B * CH * E * JB], bf16, kind="ExternalInput").ap()
    # (p, b, q, j)
    adjs = nc.dram_tensor("adjs", [128, B * CH * JB], u8, kind="ExternalInput").ap()
    # (p, b, q, f)
    hp = nc.dram_tensor("hp", [128, B * CH * H], bf16, kind="ExternalInput").ap()
    # (f, b, j)
    hsT = nc.dram_tensor("hsT", [H, B * JB], bf16, kind="ExternalInput").ap()
    WhT = nc.dram_tensor("WhT", [H, H], bf16, kind="ExternalInput").ap()
    WeT = nc.dram_tensor("WeT", [E, H], bf16, kind="ExternalInput").ap()
    UT = nc.dram_tensor("UT", [H, H], bf16, kind="ExternalInput").ap()
    wb = nc.dram_tensor("wb", [1, H], bf16, kind="ExternalInput").ap()
    ubT = nc.dram_tensor("ubT", [H, 1], f32, kind="ExternalInput").ap()
    out = nc.dram_tensor("out", [B, H, JB], f32, kind="ExternalOutput").ap()

    # DRAM bounce for the (j,e) -> [e, j] partition remap of the reduced
    # edge messages (PSUM cannot be DMA'd and engines cannot cross partitions).
    me_d = nc.dram_tensor("me_d", [B, E * JB], bf16).ap()

    EJ = E * JB  # 512: per-chunk free size of the (e, j) plane

    with tile.TileContext(nc) as tc:
        with (
            tc.tile_pool(name="const", bufs=1) as cpool,
            tc.tile_pool(name="edge", bufs=3) as epool,
            tc.tile_pool(name="masked", bufs=2) as mpool,
            tc.tile_pool(name="small", bufs=4) as spool,
            tc.tile_pool(name="pe", bufs=2, space="PSUM") as ppool_e,
            tc.tile_pool(name="pmh", bufs=2, space="PSUM") as ppool_mh,
            tc.tile_pool(name="pmsg", bufs=2, space="PSUM") as ppool_msg,
            tc.tile_pool(name="pout", bufs=2, space="PSUM") as ppool_out,
        ):
            # ---------------- mask first: it gates the whole pipeline -----
            adj_sb = cpool.tile([128, B * CH * JB], u8)
            nc.sync.dma_start(out=adj_sb[:, :], in_=adjs[:, :])
            adj_v = adj_sb.rearrange("p (b qj) -> p b qj", b=B)

            # pairwise max tree over the batch axis (adj is 0/1 so
            # max == (sum > 0))
            mt0 = cpool.tile([128, CH * JB], u8, name="mt0")
            mt1 = cpool.tile([128, CH * JB], u8, name="mt1")
            mt2 = cpool.tile([128, CH * JB], u8, name="mt2")
            mt3 = cpool.tile([128, CH * JB], u8, name="mt3")
            nc.vector.tensor_tensor(mt0[:, :], adj_v[:, 0], adj_v[:, 1], Alu.max)
            nc.vector.tensor_tensor(mt1[:, :], adj_v[:, 2], adj_v[:, 3], Alu.max)
            nc.vector.tensor_tensor(mt2[:, :], adj_v[:, 4], adj_v[:, 5], Alu.max)
            nc.vector.tensor_tensor(mt3[:, :], adj_v[:, 6], adj_v[:, 7], Alu.max)
            nc.vector.tensor_tensor(mt0[:, :], mt0[:, :], mt1[:, :], Alu.max)
            nc.vector.tensor_tensor(mt2[:, :], mt2[:, :], mt3[:, :], Alu.max)
            mask = cpool.tile([128, CH * JB], bf16)
            nc.vector.tensor_tensor(mask[:, :], mt0[:, :], mt2[:, :], Alu.max)

            # ---------------- bulk node features / weights ----------------
            h_sb = cpool.tile([128, B * CH * H], bf16)
            nc.scalar.dma_start(out=h_sb[:, :], in_=hp[:, :])
            hsT_sb = cpool.tile([H, B * JB], bf16)
            nc.scalar.dma_start(out=hsT_sb[:, :], in_=hsT[:, :])
            WhT_sb = cpool.tile([H, H], bf16)
            nc.scalar.dma_start(out=WhT_sb[:, :], in_=WhT[:, :])
            WeT_sb = cpool.tile([E, H], bf16)
            nc.scalar.dma_start(out=WeT_sb[:, :], in_=WeT[:, :])
            UT_sb = cpool.tile([H, H], bf16)
            nc.scalar.dma_start(out=UT_sb[:, :], in_=UT[:, :])
            wb_sb = cpool.tile([1, H], bf16)
            nc.scalar.dma_start(out=wb_sb[:, :], in_=wb[:, :])
            ubT_sb = cpool.tile([H, 1], f32)
            nc.scalar.dma_start(out=ubT_sb[:, :], in_=ubT[:, :])

            ones = cpool.tile([128, 1], bf16)
            nc.vector.memset(ones[:, :], 1.0)

            # deg as a row vector [1, j] = ones^T @ mask
            psum_deg = ppool_msg.tile([1, JB], f32, tag="m", name="psum_deg")
            for c in range(CH):
                nc.tensor.matmul(
                    psum_deg[:, :],
                    lhsT=ones[:, :],
                    rhs=mask[:, c * JB : (c + 1) * JB],
                    start=(c == 0),
                    stop=(c == CH - 1),
                )
            deg_sb = cpool.tile([1, JB], bf16)
            nc.scalar.copy(deg_sb[:, :], psum_deg[:, :])

            # broadcast view of the mask over the e axis (e is the middle
            # free axis of the (q, e, j) edge layout; stride-0 broadcast)
            mask_bc = mask.rearrange("p (q j) -> p q () j", q=CH).broadcast_to(
                [128, CH, E, JB]
            )

            # ---------------- per-batch software pipeline ----------------
            st = [dict() for _ in range(B)]

            def s1(b):
                # heavy streaming: edge load, mask multiply, the two
                # i-contraction matmul groups
                d = st[b]
                edge_t = epool.tile([128, CH * EJ], bf16, name="edge_t")
                nc.sync.dma_start(
                    out=edge_t[:, :],
                    in_=edge.rearrange("p (b x) -> p b x", b=B)[:, b],
                )
                masked = mpool.tile([128, CH * EJ], bf16, name="masked")
                mk_v = masked.rearrange("p (q e j) -> p q e j", q=CH, e=E)
                eg_v = edge_t.rearrange("p (q e j) -> p q e j", q=CH, e=E)
                for c in range(CH):
                    nc.vector.tensor_tensor(
                        out=mk_v[:, c],
                        in0=eg_v[:, c],
                        in1=mask_bc[:, c],
                        op=Alu.mult,
                    )
                # sum over source nodes i of the masked edge features:
                # one accumulation group, lhsT (ones) loaded once
                psum_e = ppool_e.tile([1, EJ], f32, name="psum_e")
                for c in range(CH):
                    nc.tensor.matmul(
                        psum_e[:, :],
                        lhsT=ones[:, :],
                        rhs=masked[:, c * EJ : (c + 1) * EJ],
                        start=(c == 0),
                        stop=(c == CH - 1),
                    )
                psum_mhT = ppool_mh.tile([H, JB], f32, name="psum_mhT")
                for c in range(CH):
                    nc.tensor.matmul(
                        psum_mhT[:, :],
                        lhsT=h_sb[:, (b * CH + c) * H : (b * CH + c + 1) * H],
                        rhs=mask[:, c * JB : (c + 1) * JB],
                        start=(c == 0),
                        stop=(c == CH - 1),
                    )
                d["psum_e"] = psum_e
                d["psum_mhT"] = psum_mhT

            def s2(b):
                # PSUM extraction + the (j,e) -> [e,j] remap round-trip
                d = st[b]
                me_sb = spool.tile([1, EJ], bf16, name="me_sb")
                nc.scalar.copy(me_sb[:, :], d["psum_e"][:, :])
                nc.scalar.dma_start(out=me_d[b : b + 1], in_=me_sb[0:1, :])
                me_T = spool.tile([E, JB], bf16, name="me_T")
                nc.scalar.dma_start(
                    out=me_T[:, :],
                    in_=me_d[b].rearrange("(e j) -> e j", e=E),
                )
                d["me_T"] = me_T

                mhT_s = spool.tile([H, JB], bf16, name="mhT_s")
                nc.scalar.copy(mhT_s[:, :], d["psum_mhT"][:, :])
                d["mhT_s"] = mhT_s

            def s3(b):
                # messages + update + output
                d = st[b]
                psum_msgT = ppool_msg.tile([H, JB], f32, tag="m", name="psum_msgT")
                nc.tensor.matmul(
                    psum_msgT[:, :], lhsT=WhT_sb[:, :], rhs=d["mhT_s"][:, :],
                    start=True, stop=False,
                )
                nc.tensor.matmul(
                    psum_msgT[:, :], lhsT=WeT_sb[:, :], rhs=d["me_T"][:, :],
                    start=False, stop=False,
                )
                nc.tensor.matmul(
                    psum_msgT[:, :], lhsT=wb_sb[:, :], rhs=deg_sb[:, :],
                    start=False, stop=True,
                )
                XT_s = spool.tile([H, JB], bf16, name="XT_s")
                nc.vector.tensor_tensor(
                    out=XT_s[:, :],
                    in0=psum_msgT[:, :],
                    in1=hsT_sb[:, b * JB : (b + 1) * JB],
                    op=Alu.add,
                )
                psum_outT = ppool_out.tile([H, JB], f32, tag="o", name="psum_outT")
                nc.tensor.matmul(
                    psum_outT[:, :], lhsT=UT_sb[:, :], rhs=XT_s[:, :],
                    start=True, stop=True,
                )
                out_sb = spool.tile([H, JB], f32, name="out_sb")
                nc.scalar.activation(
                    out_sb[:, :], psum_outT[:, :], ActFn.Identity,
                    bias=ubT_sb[:, :],
                )
                nc.scalar.dma_start(out=out[b], in_=out_sb[:, :])

            LAG2, LAG3 = 1, 3
            for i in range(B + LAG3):
                if i < B:
                    s1(i)
                if LAG2 <= i < B + LAG2:
                    s2(i - LAG2)
                if LAG3 <= i:
                    s3(i - LAG3)

    nc.compile()
    return nc


def _get_program():
    if "nc" not in _CACHE:
        _CACHE["nc"] = _build_program()
    return _CACHE["nc"]


def _make_in_maps(h, edge_attr, adj, W_w, W_b, U_w, U_b):
    import ml_dtypes

    bf16 = ml_dtypes.bfloat16

    h = np.asarray(h, dtype=np.float32)
    edge_attr = np.asarray(edge_attr, dtype=np.float32)
    adj = np.asarray(adj)
    W_w = np.asarray(W_w, dtype=np.float32)
    W_b = np.asarray(W_b, dtype=np.float32)
    U_w = np.asarray(U_w, dtype=np.float32)
    U_b = np.asarray(U_b, dtype=np.float32)

    # (p, b, q, f): i = 4p + q — shared by every core
    hp = np.ascontiguousarray(
        h.reshape(B, 128, CH, H).transpose(1, 0, 2, 3), dtype=bf16
    ).reshape(128, B * CH * H)
    WhT = np.ascontiguousarray(W_w[:, :H].T, dtype=bf16)
    WeT = np.ascontiguousarray(W_w[:, H:].T, dtype=bf16)
    UT = np.ascontiguousarray(U_w.T, dtype=bf16)
    wbv = np.ascontiguousarray(W_b.reshape(1, H), dtype=bf16)
    ubT = np.ascontiguousarray(U_b.reshape(H, 1), dtype=np.float32)

    in_maps = []
    for c in range(NCORES):
        j0 = c * JB
        # (p, b, q, e, j)
        ec = np.ascontiguousarray(
            edge_attr[:, :, j0 : j0 + JB, :]
            .reshape(B, 128, CH, JB, E)
            .transpose(1, 0, 2, 4, 3),
            dtype=bf16,
        ).reshape(128, B * CH * E * JB)
        # (p, b, q, j)
        ac = np.ascontiguousarray(
            adj[:, :, j0 : j0 + JB].reshape(B, 128, CH, JB).transpose(1, 0, 2, 3),
            dtype=np.uint8,
        ).reshape(128, B * CH * JB)
        # (f, b, j)
        hsT = np.ascontiguousarray(
            h[:, j0 : j0 + JB, :].transpose(2, 0, 1), dtype=bf16
        ).reshape(H, B * JB)
        in_maps.append(
            {
                "edge": ec,
                "adjs": ac,
                "hp": hp,
                "hsT": hsT,
                "WhT": WhT,
                "WeT": WeT,
                "UT": UT,
                "wb": wbv,
                "ubT": ubT,
            }
        )
    return in_maps


def _install_ntff_hook():
    """The agent image lacks antenv.axon_hooks; synthesize it so trace=True
    can reach the libaxon NTFF profiling entry points."""
    import sys
    import types

    try:
        from antenv.axon_hooks import get_axon_ntff_profile_hook  # noqa: F401

        return
    except ImportError:
        pass
    import antenv

    mod = types.ModuleType("antenv.axon_hooks")
    _h = [None]
    mod.set_axon_ntff_profile_hook = lambda hook: _h.__setitem__(0, hook)
    mod.get_axon_ntff_profile_hook = lambda: _h[0]
    sys.modules["antenv.axon_hooks"] = mod
    antenv.axon_hooks = mod
    try:
        from trn_agent_boot.trn_boot import _ntff_profile_via_ctypes

        mod.set_axon_ntff_profile_hook(
            _ntff_profile_via_ctypes("/opt/axon/libaxon_pjrt.so")
        )
    except Exception:
        pass
    # avoid the bucket upload (no bucket in this container)
    import concourse.bass_utils as bu

    bu.upload_artifacts = lambda tmpdir: str(tmpdir)


def run(h, edge_attr, adj, W_w, W_b, U_w, U_b, trace=False, trace_cores=None):
    """Run the kernel; returns (output, BassKernelResults)."""
    _ensure_path()
    if trace:
        _install_ntff_hook()
    from concourse.bass_utils import run_bass_kernel_spmd

    nc = _get_program()
    in_maps = _make_in_maps(h, edge_attr, adj, W_w, W_b, U_w, U_b)
    kw = {}
    if trace:
        kw = {"trace": True, "trace_cores": trace_cores or [0]}
    res = run_bass_kernel_spmd(nc, in_maps, list(range(NCORES)), **kw)
    outs = [res.results[c]["out"].transpose(0, 2, 1) for c in range(NCORES)]
    full = np.concatenate(outs, axis=1)  # [B, N, H]
    return full, res


def kernel(h, edge_attr, adj, W_w, W_b, U_w, U_b):
    full, _ = run(h, edge_attr, adj, W_w, W_b, U_w, U_b)
    return full


# revision 7
# speedup vs baseline: 2.0661x; 1.2764x over previous
"""DMPNN layer kernel for 8 Trainium2 NeuronCores.

Sharding: data-parallel over destination nodes j (dim 2 of edge_attr/adj,
dim 1 of the output). Each core gets a 64-column j-slice of edge_attr/adj,
the full h (needed because messages sum over all source nodes i), and the
small weights replicated. The batch-global mask (adj.sum(0) > 0) only needs
the core's own j-slice of adj over the full batch, so no collective at all.
adj ships bit-packed along the batch axis (one byte per (i,j), lossless);
the device reduces it with a single byte!=0 compare, which is exactly
max_b adj[b,i,j] for 0/1 entries.

Numerics: the correctness gate is rel_err < 2e-2 (norm-relative); staging
edge_attr/h/weights as bf16 keeps the result at ~3.6e-3 while halving HBM
traffic and doubling PE/DVE throughput. All matmuls accumulate in fp32
PSUM; the mask multiply is exact (mask is 0.0/1.0 in bf16).

Structure per core (source nodes i = 4p + q, j in the core's 64-col slice):
  mask[i,j]   = (packed_adj[i,j] != 0)
  me[b,j,e]   = sum_i mask[i,j] edge[b,i,j,e]     (DVE mask-mult + PE ones-
                contraction on column-group 3, concurrent with the mh MMs)
  mh+[.,b]    = [h|1]^T_chunk @ mask_chunk  -> [65, j]: rows 0-63 = mh^T,
                row 64 = deg (the ones column makes deg ride along free)
  msgT        = [WhT; wb]^T @ mh+  + We^T @ me^T  (wb*deg folds into one MM)
  outT        = U^T @ (msgT + hsT) + ub           (ub as ACT bias)

s3 runs in two groups of 4 batches on [64, 256] PSUM tiles. Group 0 remaps
me (j,e)->[e,j] through a small DRAM bounce (overlapped mid-stream); group 1
instead uses 8 rank-1 matmuls straight from the [1,(b e j)] SBUF copy,
keeping the kernel tail free of a DMA round-trip. Small DMAs issue from
GPSIMD (SWDGE, ~25ns engine cost) so the ACT/SP queues stay unblocked.
"""

import numpy as np


def _ensure_path():
    try:
        import concourse.bass  # noqa: F401
    except ImportError:
        import sys

        for p in ("/opt/trn_rl_repo", "/root/.axon_site/_ro/trn_rl_repo"):
            if p not in sys.path:
                sys.path.insert(0, p)


B, N, H, E = 8, 512, 64, 8
NCORES = 8
JB = N // NCORES  # 64 destination columns per core
CH = N // 128  # 4 source-node sub-chunks (i = 4p + q)
HA = H + 1  # h augmented with a ones column (deg rides the mh matmul)
G = 4  # batches per s3 group
EJ = E * JB  # 512

_CACHE = {}


def _build_program(ones_colgroup=True):
    _ensure_path()
    import concourse.bacc as bacc
    import concourse.mybir as mybir
    import concourse.tile as tile

    dt = mybir.dt
    f32 = dt.float32
    bf16 = dt.bfloat16
    u8 = dt.uint8
    Alu = mybir.AluOpType
    ActFn = mybir.ActivationFunctionType

    nc = bacc.Bacc("TRN2", debug=False, num_devices=NCORES)

    # (p, b, q, e, j) — per-partition contiguous 4 KiB per batch
    edge = nc.dram_tensor("edge", [128, B * CH * EJ], bf16, kind="ExternalInput").ap()
    # (p, q, j): adj bit-packed along the batch axis
    adjp = nc.dram_tensor("adjp", [128, CH * JB], u8, kind="ExternalInput").ap()
    # (p, b, q, f+1): h with a trailing ones column per chunk
    hp = nc.dram_tensor("hp", [128, B * CH * HA], bf16, kind="ExternalInput").ap()
    # (f, b, j)
    hsT = nc.dram_tensor("hsT", [H, B * JB], bf16, kind="ExternalInput").ap()
    # [WhT; wb] stacked: [65, 64]
    Whb = nc.dram_tensor("Whb", [HA, H], bf16, kind="ExternalInput").ap()
    WeT = nc.dram_tensor("WeT", [E, H], bf16, kind="ExternalInput").ap()
    # We^T flattened e-major on one partition: [1, E*H]
    Wef = nc.dram_tensor("Wef", [1, E * H], bf16, kind="ExternalInput").ap()
    UT = nc.dram_tensor("UT", [H, H], bf16, kind="ExternalInput").ap()
    ubT = nc.dram_tensor("ubT", [H, 1], f32, kind="ExternalInput").ap()
    out = nc.dram_tensor("out", [B, H, JB], f32, kind="ExternalOutput").ap()

    # DRAM bounce for group 0's (j,e) -> [e,j] partition remap
    me_d = nc.dram_tensor("me_d", [G, EJ], bf16).ap()

    PE_ROW = 96 if ones_colgroup else 0

    with tile.TileContext(nc) as tc:
        with (
            tc.tile_pool(name="const", bufs=1) as cpool,
            tc.tile_pool(name="edge", bufs=3) as epool,
            tc.tile_pool(name="masked", bufs=2) as mpool,
            tc.tile_pool(name="mh4", bufs=2) as mh4pool,
            tc.tile_pool(name="me4", bufs=2) as me4pool,
            tc.tile_pool(name="xt", bufs=2) as xtpool,
            tc.tile_pool(name="outp", bufs=2) as outpool,
            tc.tile_pool(name="pe", bufs=2, space="PSUM") as ppool_e,
            tc.tile_pool(name="pmh", bufs=2, space="PSUM") as ppool_mh,
            tc.tile_pool(name="pmsg", bufs=2, space="PSUM") as ppool_msg,
            tc.tile_pool(name="pout", bufs=2, space="PSUM") as ppool_out,
        ):
            # ---------------- mask first: it gates the whole pipeline -----
            adj_sb = cpool.tile([128, CH * JB], u8)
            nc.sync.dma_start(out=adj_sb[:, :], in_=adjp[:, :])
            mask = cpool.tile([128, CH * JB], bf16)
            nc.vector.tensor_scalar(
                out=mask[:, :], in0=adj_sb[:, :], scalar1=0, scalar2=None,
                op0=Alu.not_equal,
            )

            # ---------------- bulk node features / weights (SWDGE) --------
            h_sb = cpool.tile([128, B * CH * HA], bf16)
            nc.gpsimd.dma_start(out=h_sb[:, :], in_=hp[:, :])
            hsT_sb = cpool.tile([H, B * JB], bf16)
            nc.gpsimd.dma_start(out=hsT_sb[:, :], in_=hsT[:, :])
            Whb_sb = cpool.tile([HA, H], bf16)
            nc.gpsimd.dma_start(out=Whb_sb[:, :], in_=Whb[:, :])
            WeT_sb = cpool.tile([E, H], bf16)
            nc.gpsimd.dma_start(out=WeT_sb[:, :], in_=WeT[:, :])
            Wef_sb = cpool.tile([1, E * H], bf16)
            nc.gpsimd.dma_start(out=Wef_sb[:, :], in_=Wef[:, :])
            UT_sb = cpool.tile([H, H], bf16)
            nc.gpsimd.dma_start(out=UT_sb[:, :], in_=UT[:, :])
            ubT_sb = cpool.tile([H, 1], f32)
            nc.gpsimd.dma_start(out=ubT_sb[:, :], in_=ubT[:, :])

            ones = cpool.tile([128, 1], bf16)
            nc.vector.memset(ones[:, :], 1.0)

            # broadcast view of the mask over the e axis (middle free axis)
            mask_bc = mask.rearrange("p (q j) -> p q () j", q=CH).broadcast_to(
                [128, CH, E, JB]
            )

            st = [dict() for _ in range(B)]
            grp = [dict() for _ in range(2)]

            def s1(b):
                # heavy streaming: edge load, one fused mask multiply, the
                # two i-contraction matmul groups (ones-reduce on column
                # group 3, concurrent with the mh group on columns 0-64)
                d = st[b]
                edge_t = epool.tile([128, CH * EJ], bf16, name="edge_t")
                nc.sync.dma_start(
                    out=edge_t[:, :],
                    in_=edge.rearrange("p (b x) -> p b x", b=B)[:, b],
                )
                masked = mpool.tile([128, CH * EJ], bf16, name="masked")
                nc.vector.tensor_tensor(
                    out=masked.rearrange("p (q e j) -> p q e j", q=CH, e=E),
                    in0=edge_t.rearrange("p (q e j) -> p q e j", q=CH, e=E),
                    in1=mask_bc,
                    op=Alu.mult,
                )
                psum_e = ppool_e.tile([128, EJ], f32, name="psum_e")
                for c in range(CH):
                    nc.tensor.matmul(
                        psum_e[PE_ROW : PE_ROW + 1, :],
                        lhsT=ones[:, :],
                        rhs=masked[:, c * EJ : (c + 1) * EJ],
                        start=(c == 0),
                        stop=(c == CH - 1),
                        tile_position=(0, PE_ROW) if ones_colgroup else None,
                    )
                psum_mh = ppool_mh.tile([HA, JB], f32, name="psum_mh")
                for c in range(CH):
                    nc.tensor.matmul(
                        psum_mh[:, :],
                        lhsT=h_sb[:, (b * CH + c) * HA : (b * CH + c + 1) * HA],
                        rhs=mask[:, c * JB : (c + 1) * JB],
                        start=(c == 0),
                        stop=(c == CH - 1),
                    )
                d["psum_e"] = psum_e
                d["psum_mh"] = psum_mh

            def s2(b):
                # PSUM extraction into the group tiles
                d = st[b]
                g, slot = divmod(b, G)
                if slot == 0:
                    grp[g]["mh4"] = mh4pool.tile([HA, G * JB], bf16, name="mh4")
                    grp[g]["me4"] = me4pool.tile([1, G * EJ], bf16, name="me4")
                nc.scalar.copy(
                    grp[g]["me4"][0:1, slot * EJ : (slot + 1) * EJ],
                    d["psum_e"][PE_ROW : PE_ROW + 1, :],
                )
                nc.scalar.copy(
                    grp[g]["mh4"][:, slot * JB : (slot + 1) * JB],
                    d["psum_mh"][:, :],
                )
                if g == 0 and slot == G - 1:
                    # group 0: bounce through DRAM to land me as [e, (b j)]
                    nc.gpsimd.dma_start(
                        out=me_d.rearrange("b x -> () (b x)"),
                        in_=grp[0]["me4"][0:1, :],
                    )
                    me_T = me4pool.tile([E, G * JB], bf16, name="me_T")
                    nc.gpsimd.dma_start(
                        out=me_T.rearrange("e (b j) -> e b j", b=G),
                        in_=me_d.rearrange("b (e j) -> e b j", e=E),
                    )
                    grp[0]["me_T"] = me_T

            def s3(g):
                # messages + update + output for a group of 4 batches
                psum_msg = ppool_msg.tile([H, G * JB], f32, name="psum_msg")
                nc.tensor.matmul(
                    psum_msg[:, :], lhsT=Whb_sb[:, :], rhs=grp[g]["mh4"][:, :],
                    start=True, stop=False,
                )
                if g == 0:
                    nc.tensor.matmul(
                        psum_msg[:, :], lhsT=WeT_sb[:, :], rhs=grp[0]["me_T"][:, :],
                        start=False, stop=True,
                    )
                else:
                    # tail group: 8 rank-1 matmuls straight from SBUF, no
                    # DMA round-trip on the critical tail
                    me4v = grp[g]["me4"].rearrange(
                        "p (b e j) -> p b e j", b=G, e=E
                    )
                    for e in range(E):
                        nc.tensor.matmul(
                            psum_msg[:, :],
                            lhsT=Wef_sb[0:1, e * H : (e + 1) * H],
                            rhs=me4v[:, :, e],
                            start=False,
                            stop=(e == E - 1),
                        )
                XT = xtpool.tile([H, G * JB], bf16, name="XT")
                nc.vector.tensor_tensor(
                    out=XT[:, :],
                    in0=psum_msg[:, :],
                    in1=hsT_sb[:, g * G * JB : (g + 1) * G * JB],
                    op=Alu.add,
                )
                psum_out = ppool_out.tile([H, G * JB], f32, name="psum_out")
                nc.tensor.matmul(
                    psum_out[:, :], lhsT=UT_sb[:, :], rhs=XT[:, :],
                    start=True, stop=True,
                )
                out_sb = outpool.tile([H, G * JB], f32, name="out_sb")
                nc.scalar.activation(
                    out_sb[:, :], psum_out[:, :], ActFn.Identity,
                    bias=ubT_sb[:, :],
                )
                nc.gpsimd.dma_start(
                    out=out[g * G : (g + 1) * G].rearrange("b h j -> h b j"),
                    in_=out_sb.rearrange("h (b j) -> h b j", b=G),
                )

            # software pipeline: s2 lags s1 by 1; group-0 s3 two iterations
            # after its bounce store/load; group-1 s3 right after s2(7)
            for i in range(B + 1):
                if i < B:
                    s1(i)
                if i >= 1:
                    s2(i - 1)
                if i == 6:
                    s3(0)
            s3(1)

    nc.compile()
    return nc


def _get_program():
    if "nc" not in _CACHE:
        _CACHE["nc"] = _build_program()
    return _CACHE["nc"]


def _make_in_maps(h, edge_attr, adj, W_w, W_b, U_w, U_b):
    import ml_dtypes

    bf16 = ml_dtypes.bfloat16

    h = np.asarray(h, dtype=np.float32)
    edge_attr = np.asarray(edge_attr, dtype=np.float32)
    adj = np.asarray(adj)
    W_w = np.asarray(W_w, dtype=np.float32)
    W_b = np.asarray(W_b, dtype=np.float32)
    U_w = np.asarray(U_w, dtype=np.float32)
    U_b = np.asarray(U_b, dtype=np.float32)

    # (p, b, q, f+1): i = 4p + q, trailing ones column per chunk
    hb = np.ascontiguousarray(
        h.reshape(B, 128, CH, H).transpose(1, 0, 2, 3), dtype=bf16
    )
    hp = np.concatenate([hb, np.ones((128, B, CH, 1), dtype=bf16)], axis=3).reshape(
        128, B * CH * HA
    )
    Whb = np.ascontiguousarray(
        np.vstack([W_w[:, :H].T, W_b.reshape(1, H)]), dtype=bf16
    )
    WeT = np.ascontiguousarray(W_w[:, H:].T, dtype=bf16)
    Wef = WeT.reshape(1, E * H).copy()
    UT = np.ascontiguousarray(U_w.T, dtype=bf16)
    ubT = np.ascontiguousarray(U_b.reshape(H, 1), dtype=np.float32)

    # bit-pack adj along the batch axis: byte != 0  <=>  max_b adj[b,i,j]
    adj_packed = np.packbits(adj.astype(bool), axis=0)[0]  # [N, N] uint8

    in_maps = []
    for c in range(NCORES):
        j0 = c * JB
        # (p, b, q, e, j)
        ec = np.ascontiguousarray(
            edge_attr[:, :, j0 : j0 + JB, :]
            .reshape(B, 128, CH, JB, E)
            .transpose(1, 0, 2, 4, 3),
            dtype=bf16,
        ).reshape(128, B * CH * EJ)
        # (p, q, j)
        ac = np.ascontiguousarray(
            adj_packed[:, j0 : j0 + JB].reshape(128, CH, JB)
        ).reshape(128, CH * JB)
        # (f, b, j)
        hsT = np.ascontiguousarray(
            h[:, j0 : j0 + JB, :].transpose(2, 0, 1), dtype=bf16
        ).reshape(H, B * JB)
        in_maps.append(
            {
                "edge": ec,
                "adjp": ac,
                "hp": hp,
                "hsT": hsT,
                "Whb": Whb,
                "WeT": WeT,
                "Wef": Wef,
                "UT": UT,
                "ubT": ubT,
            }
        )
    return in_maps


def _install_ntff_hook():
    """The agent image lacks antenv.axon_hooks; synthesize it so trace=True
    can reach the libaxon NTFF profiling entry points."""
    import sys
    import types

    try:
        from antenv.axon_hooks import get_axon_ntff_profile_hook  # noqa: F401

        return
    except ImportError:
        pass
    import antenv

    mod = types.ModuleType("antenv.axon_hooks")
    _h = [None]
    mod.set_axon_ntff_profile_hook = lambda hook: _h.__setitem__(0, hook)
    mod.get_axon_ntff_profile_hook = lambda: _h[0]
    sys.modules["antenv.axon_hooks"] = mod
    antenv.axon_hooks = mod
    try:
        from trn_agent_boot.trn_boot import _ntff_profile_via_ctypes

        mod.set_axon_ntff_profile_hook(
            _ntff_profile_via_ctypes("/opt/axon/libaxon_pjrt.so")
        )
    except Exception:
        pass
    # avoid the bucket upload (no bucket in this container)
    import concourse.bass_utils as bu

    bu.upload_artifacts = lambda tmpdir: str(tmpdir)


def run(h, edge_attr, adj, W_w, W_b, U_w, U_b, trace=False, trace_cores=None):
    """Run the kernel; returns (output, BassKernelResults)."""
    _ensure_path()
    if trace:
        _install_ntff_hook()
    from concourse.bass_utils import run_bass_kernel_spmd

    nc = _get_program()
    in_maps = _make_in_maps(h, edge_attr, adj, W_w, W_b, U_w, U_b)
    kw = {}
    if trace:
        kw = {"trace": True, "trace_cores": trace_cores or [0]}
    res = run_bass_kernel_spmd(nc, in_maps, list(range(NCORES)), **kw)
    outs = [res.results[c]["out"].transpose(0, 2, 1) for c in range(NCORES)]
    full = np.concatenate(outs, axis=1)  # [B, N, H]
    return full, res


def kernel(h, edge_attr, adj, W_w, W_b, U_w, U_b):
    full, _ = run(h, edge_attr, adj, W_w, W_b, U_w, U_b)
    return full
